# revision 14
# baseline (speedup 1.0000x reference)
"""CRPS loss kernel for Trainium2 (8 NeuronCores, batch-parallel).

Math (per grid point, N=32 ensemble members x_i, target y):
  term1 = (1/N) sum_i |x_i - y|
  term2 = (1/N^2) sum_i (2i+1-N) x_sorted_i          (reference sorts)
        = (1/N^2) sum_{i<j} |x_i - x_j|
        = (1/N^2) (2 sum_{i<j} max(x_i, x_j) - (N-1) sum_i x_i)
  CRPS  = term1 - term2
The latitude weight w_h > 0 multiplies both p and y in the reference, so
it factors out of every term; the device reduces raw per-latitude sums
and the host applies w_h and the final mean in float64.  sum_i x_i is a
plain linear reduction of the input, done on the host in f64.

Sharding: pure data parallel over B=16 (2 batches per core).  Per-core
SBUF layout [h=121 partitions, b=2, n=32, w=240] bf16 (host pre-cast;
halves DMA).  The O(N^2) pairwise-max sum = 31 shifted tensor_max ops on
the vector engine (bf16 2x, measured 0.44 ns/elem-lane).  Accumulation
is split by measured rates: shifts d in ACT_DS are consumed by the
scalar engine (activation Copy accum_out, 0.58 ns/elem) via a 2-slot
ring; shifts in DVE_DS are tensor_add'ed by the vector engine itself.
Ring production and DVE's own add work are interleaved so neither
engine idles; term1 rides ACT (DVE subtract -> ACT Abs accum).

Outputs per core: [121, 2] f32 rows {sum|x-y|, sum pairwise max} per
latitude; host combines with the f64 input sum.
"""

import numpy as np
import ml_dtypes

import concourse.bass as bass
import concourse.mybir as mybir
from concourse.bass_utils import run_bass_kernel_spmd

H, W, B, N = 121, 240, 16, 32
N_CORES = 8
B_LOC = B // N_CORES

F32 = mybir.dt.float32
BF16 = mybir.dt.bfloat16
FP8 = mybir.dt.float8e4
ALU = mybir.AluOpType
AFT = mybir.ActivationFunctionType

# measured-rate balance: ACT consumes d=1..20, DVE adds d=21..31 itself
ACT_DS = list(range(1, 21))
DVE_DS = list(range(21, N))

_NC_CACHE = {}


def build_nc(repeat=1, detect_races=True):
    """repeat>1 replicates the whole compute phase for slope timing."""
    key = (repeat, detect_races)
    if key in _NC_CACHE:
        return _NC_CACHE[key]
    nc = bass.Bass(detect_race_conditions=detect_races)
    x_in = nc.declare_dram_parameter("x", [H, B_LOC * N * W], BF16, isOutput=False)
    y_in = nc.declare_dram_parameter("y", [H, B_LOC * W], BF16, isOutput=False)
    o_out = nc.declare_dram_parameter("o", [H, 2], F32, isOutput=True)

    NA = len(ACT_DS)                 # 20 ring consumes per iteration
    SPI = NA + 1                     # s_sem incs per iteration (+1 acc-reduce)
    RING = 3
    ring_n = N - ACT_DS[0]           # 31

    # own-work items fill DVE's ring-stall gaps (sub + DVE_DS max/adds)
    own_items = [("sub", None)] + [("own", d) for d in DVE_DS[1:]]

    with (
        nc.sbuf_tensor([H, B_LOC, N, W], BF16) as xt,
        nc.sbuf_tensor([H, B_LOC, W], BF16) as yt,
        nc.sbuf_tensor([H, B_LOC, N - DVE_DS[0], W], BF16) as acc,
        nc.sbuf_tensor([H, B_LOC, N - DVE_DS[0] - 1, W], BF16) as mxd,
        nc.sbuf_tensor([H, B_LOC, ring_n, W], BF16) as mxa,
        nc.sbuf_tensor([H, B_LOC, ring_n - 1, W], BF16) as mxb,
        nc.sbuf_tensor([H, B_LOC, ring_n - 2, W], BF16) as mxc,
        nc.sbuf_tensor([H, B_LOC, N, W], BF16) as dif,
        nc.sbuf_tensor([H, B_LOC, N, W], FP8) as dump,
        nc.sbuf_tensor([H, NA + 1, 1], F32) as at,
        nc.sbuf_tensor([H, NA + 1], F32) as at_sink,
        nc.sbuf_tensor([H, 1], F32) as a1,
        nc.sbuf_tensor([H, 1], F32) as ot_a2,
        nc.sbuf_tensor([H, 2], F32) as ot,
        nc.semaphore() as dma_sem,
        nc.semaphore() as v_sem,
        nc.semaphore() as s_sem,
        nc.Block() as block,
    ):
        xv = xt[:]
        av = acc[:]
        ring = [mxa[:], mxb[:], mxc[:]]

        @block.sync
        def _(sync):
            sync.dma_start(
                out=xt[:],
                in_=x_in[:].rearrange("h (b n w) -> h b n w", b=B_LOC, n=N, w=W),
            ).then_inc(dma_sem, 16)
            sync.dma_start(
                out=yt[:],
                in_=y_in[:].rearrange("h (b w) -> h b w", b=B_LOC, w=W),
            ).then_inc(dma_sem, 16)
            sync.wait_ge(s_sem, repeat * SPI + 1)
            sync.dma_start(out=o_out[:], in_=ot[:]).then_inc(dma_sem, 16)

        @block.vector
        def _(vector):
            vector.wait_ge(dma_sem, 32)
            ybc = yt[:].unsqueeze(2).broadcast_to((H, B_LOC, N, W))

            def do_own(item):
                kind, d = item
                if kind == "sub":
                    nc.vector.tensor_tensor(
                        dif[:], xv, ybc, op=ALU.subtract
                    ).then_inc(v_sem, 1)
                else:
                    nc.vector.tensor_max(
                        mxd[:, :, : N - d, :], xv[:, :, d:, :], xv[:, :, : N - d, :]
                    )
                    nc.vector.tensor_add(
                        av[:, :, : N - d, :], av[:, :, : N - d, :],
                        mxd[:, :, : N - d, :],
                    )

            for it in range(repeat):
                vb = (2 + NA) * it
                sb = SPI * it
                if it > 0:
                    # ACT past previous Abs once its first ring consume lands
                    vector.wait_ge(s_sem, SPI * (it - 1) + 1)
                do_own(own_items[0])  # sub -> v_sem vb+1
                if it > 0:
                    # first own-d overwrites acc: previous acc-reduce done
                    vector.wait_ge(s_sem, SPI * it)
                d0 = DVE_DS[0]
                nc.vector.tensor_max(
                    av[:, :, : N - d0, :], xv[:, :, d0:, :], xv[:, :, : N - d0, :]
                )
                oi = 1
                for k, d in enumerate(ACT_DS):
                    if k >= RING:
                        vector.wait_ge(s_sem, sb + k - RING + 1)
                    nc.vector.tensor_max(
                        ring[k % RING][:, :, : N - d, :],
                        xv[:, :, d:, :],
                        xv[:, :, : N - d, :],
                    ).then_inc(v_sem, 1)  # vb+2+k
                    if k % 2 == 1 and oi < len(own_items):
                        do_own(own_items[oi])
                        oi += 1
                while oi < len(own_items):
                    do_own(own_items[oi])
                    oi += 1
                nc.vector.memset(ot_a2[:], 0.0).then_inc(v_sem, 1)  # vb+2+NA

        @block.scalar
        def _(scalar):
            for it in range(repeat):
                vb = (2 + NA) * it
                scalar.wait_ge(v_sem, vb + 1)
                nc.scalar.activation(dump[:], dif[:], AFT.Abs, accum_out=a1[:])
                for k, d in enumerate(ACT_DS):
                    scalar.wait_ge(v_sem, vb + 2 + k)
                    nc.scalar.activation(
                        dump[:, :, : N - d, :],
                        ring[k % RING][:, :, : N - d, :],
                        AFT.Copy,
                        accum_out=at[:, k + 1, :],
                    ).then_inc(s_sem, 1)
                scalar.wait_ge(v_sem, vb + 2 + NA)  # all DVE adds done
                nc.scalar.activation(
                    dump[:, :, : N - DVE_DS[0], :], av, AFT.Copy,
                    accum_out=at[:, 0, :]
                ).then_inc(s_sem, 1)
            nc.scalar.activation(
                at_sink[:], at[:].squeeze(2), AFT.Copy, accum_out=ot_a2[:]
            )
            nc.scalar.copy(ot[:, 0:1], a1[:])
            nc.scalar.copy(ot[:, 1:2], ot_a2[:]).then_inc(s_sem, 1)

    _NC_CACHE[key] = nc
    return nc


def _prep_inputs(predictions, targets):
    """Full f32 [B,N,H,W]/[B,H,W] -> per-core bf16 maps, layout [h,b,n,w]."""
    p = np.asarray(predictions, dtype=np.float32)
    t = np.asarray(targets, dtype=np.float32)
    pt = np.ascontiguousarray(p.transpose(2, 0, 1, 3)).astype(ml_dtypes.bfloat16)
    tt = np.ascontiguousarray(t.transpose(1, 0, 2)).astype(ml_dtypes.bfloat16)
    in_maps = []
    for c in range(N_CORES):
        xc = np.ascontiguousarray(pt[:, B_LOC * c : B_LOC * (c + 1)]).reshape(
            H, B_LOC * N * W
        )
        yc = np.ascontiguousarray(tt[:, B_LOC * c : B_LOC * (c + 1)]).reshape(
            H, B_LOC * W
        )
        in_maps.append({"x": xc, "y": yc})
    return in_maps


def _lat_weights_f64():
    lats = np.arange(90.0, -91.5, -1.5)  # [121]
    w = np.cos(np.deg2rad(lats))
    return H * (w / np.sum(w))


def _combine(outs, predictions):
    """outs: list of [H,2] f32 -> scalar f32 (host math in f64)."""
    w = _lat_weights_f64()
    p = np.asarray(predictions, dtype=np.float32)
    pb = p.astype(ml_dtypes.bfloat16).astype(np.float64)  # match device rounding
    a3_h = pb.sum(axis=(0, 1, 3))  # [H]
    a1_h = np.zeros(H, np.float64)
    a2_h = np.zeros(H, np.float64)
    for o in outs:
        o = np.asarray(o, dtype=np.float64)
        a1_h += o[:, 0]
        a2_h += o[:, 1]
    s2 = 2.0 * a2_h - (N - 1) * a3_h
    crps_h = a1_h / N - s2 / (N * N)
    total = float(np.dot(w, crps_h))
    return np.float32(total / (B * H * W))


def kernel(predictions, targets):
    nc = build_nc()
    in_maps = _prep_inputs(predictions, targets)
    res = run_bass_kernel_spmd(nc, in_maps, list(range(N_CORES)))
    outs = [res.results[i]["o"] for i in range(N_CORES)]
    return _combine(outs, predictions)


# revision 15
# speedup vs baseline: 1.0581x; 1.0581x over previous
"""CRPS loss kernel for Trainium2 (8 NeuronCores, batch-parallel).

Math (per grid point, N=32 ensemble members x_i, target y):
  term1 = (1/N) sum_i |x_i - y|
  term2 = (1/N^2) sum_i (2i+1-N) x_sorted_i          (reference sorts)
        = (1/N^2) sum_{i<j} |x_i - x_j|
        = (1/N^2) (2 sum_{i<j} max(x_i, x_j) - (N-1) sum_i x_i)
  CRPS  = term1 - term2
The latitude weight w_h > 0 multiplies both p and y in the reference, so
it factors out of every term; the device reduces raw per-latitude sums
and the host applies w_h and the final mean in float64.  sum_i x_i is a
plain linear reduction of the input, done on the host in f64.

Sharding: pure data parallel over B=16 (2 batches per core).  Per-core
SBUF layout [h=121 partitions, b=2, n=32, w=240] bf16 (host pre-cast;
halves DMA).  The O(N^2) pairwise-max sum = 31 shifted tensor_max ops on
the vector engine (bf16 2x, measured 0.44 ns/elem-lane).  Accumulation
is split by measured rates: shifts d in ACT_DS are consumed by the
scalar engine (activation Copy accum_out, 0.58 ns/elem) via a 2-slot
ring; shifts in DVE_DS are tensor_add'ed by the vector engine itself.
Ring production and DVE's own add work are interleaved so neither
engine idles; term1 rides ACT (DVE subtract -> ACT Abs accum).

Outputs per core: [121, 2] f32 rows {sum|x-y|, sum pairwise max} per
latitude; host combines with the f64 input sum.
"""

import numpy as np
import ml_dtypes

import concourse.bass as bass
import concourse.mybir as mybir
from concourse.bass_utils import run_bass_kernel_spmd

H, W, B, N = 121, 240, 16, 32
N_CORES = 8
B_LOC = B // N_CORES

F32 = mybir.dt.float32
BF16 = mybir.dt.bfloat16
FP8 = mybir.dt.float8e4
ALU = mybir.AluOpType
AFT = mybir.ActivationFunctionType

# shift pairing: (d, 33-d) fills a uniform [b, 31, w] slot (32-d + d-1 = 31
# elements).  ACT consumes 13 pairs via a 2-slot ring (one activation per
# pair); DVE handles d=1 (accumulator init) plus 2 pairs itself.
ACT_PAIRS = [(d, 33 - d) for d in range(2, 15)]   # 13 pairs
DVE_PAIRS = [(15, 18), (16, 17)]

_NC_CACHE = {}


def build_nc(repeat=1, detect_races=True):
    """repeat>1 replicates the whole compute phase for slope timing."""
    key = (repeat, detect_races)
    if key in _NC_CACHE:
        return _NC_CACHE[key]
    nc = bass.Bass(detect_race_conditions=detect_races)
    x_in = nc.declare_dram_parameter("x", [H, B_LOC * N * W], BF16, isOutput=False)
    y_in = nc.declare_dram_parameter("y", [H, B_LOC * W], BF16, isOutput=False)
    o_out = nc.declare_dram_parameter("o", [H, 2], F32, isOutput=True)

    NP = len(ACT_PAIRS)              # 13 ring consumes per iteration
    SPI = NP + 1                     # s_sem incs per iteration (+1 acc-reduce)
    RING = 2
    V = NP + 2                       # v_sem incs per iteration

    with (
        nc.sbuf_tensor([H, B_LOC, N, W], BF16) as xt,
        nc.sbuf_tensor([H, B_LOC, W], BF16) as yt,
        nc.sbuf_tensor([H, B_LOC, N - 1, W], BF16) as acc,
        nc.sbuf_tensor([H, B_LOC, N - DVE_PAIRS[0][0], W], BF16) as mxd,
        nc.sbuf_tensor([H, B_LOC, N - 1, W], BF16) as mxa,
        nc.sbuf_tensor([H, B_LOC, N - 1, W], BF16) as mxb,
        nc.sbuf_tensor([H, B_LOC, N, W], BF16) as dif,
        nc.sbuf_tensor([H, B_LOC, N - 1, W], FP8) as dump,
        nc.sbuf_tensor([H, NP + 1, 1], F32) as at,
        nc.sbuf_tensor([H, NP + 1], F32) as at_sink,
        nc.sbuf_tensor([H, 1], F32) as a1,
        nc.sbuf_tensor([H, 1], F32) as ot_a2,
        nc.sbuf_tensor([H, 2], F32) as ot,
        nc.sbuf_tensor([H, B_LOC, N, W], FP8) as dump_dif,
        nc.semaphore() as dma_sem,
        nc.semaphore() as v_sem,
        nc.semaphore() as s_sem,
        nc.Block() as block,
    ):
        xv = xt[:]
        av = acc[:]
        ring = [mxa[:], mxb[:]]

        @block.sync
        def _(sync):
            sync.dma_start(
                out=xt[:],
                in_=x_in[:].rearrange("h (b n w) -> h b n w", b=B_LOC, n=N, w=W),
            ).then_inc(dma_sem, 16)
            sync.dma_start(
                out=yt[:],
                in_=y_in[:].rearrange("h (b w) -> h b w", b=B_LOC, w=W),
            ).then_inc(dma_sem, 16)
            sync.wait_ge(s_sem, repeat * SPI + 1)
            sync.dma_start(out=o_out[:], in_=ot[:]).then_inc(dma_sem, 16)

        @block.vector
        def _(vector):
            vector.wait_ge(dma_sem, 32)
            ybc = yt[:].unsqueeze(2).broadcast_to((H, B_LOC, N, W))

            # own-work queue: sub first, then DVE pairs as (max, add) chunks
            def own_gen():
                yield ("sub", None, None)
                for da, db in DVE_PAIRS:
                    yield ("max", da, None)
                    yield ("add", da, None)
                    yield ("max", db, None)
                    yield ("add", db, None)

            def do_own(item):
                kind, d, _ = item
                if kind == "sub":
                    nc.vector.tensor_tensor(
                        dif[:], xv, ybc, op=ALU.subtract
                    ).then_inc(v_sem, 1)
                elif kind == "max":
                    nc.vector.tensor_max(
                        mxd[:, :, : N - d, :], xv[:, :, d:, :], xv[:, :, : N - d, :]
                    )
                else:
                    nc.vector.tensor_add(
                        av[:, :, : N - d, :], av[:, :, : N - d, :],
                        mxd[:, :, : N - d, :],
                    )

            for it in range(repeat):
                vb = V * it
                sb = SPI * it
                own = list(own_gen())
                if it > 0:
                    vector.wait_ge(s_sem, SPI * (it - 1) + 1)
                do_own(own[0])  # sub -> v_sem vb+1
                if it > 0:
                    # d=1 overwrites acc: previous acc-reduce must be done
                    vector.wait_ge(s_sem, SPI * it)
                nc.vector.tensor_max(
                    av[:, :, : N - 1, :], xv[:, :, 1:, :], xv[:, :, : N - 1, :]
                )
                oi = 1
                for k, (da, db) in enumerate(ACT_PAIRS):
                    if k >= RING:
                        vector.wait_ge(s_sem, sb + k - RING + 1)
                    slot = ring[k % RING]
                    nc.vector.tensor_max(
                        slot[:, :, : N - da, :],
                        xv[:, :, da:, :],
                        xv[:, :, : N - da, :],
                    )
                    nc.vector.tensor_max(
                        slot[:, :, N - da : N - 1, :],
                        xv[:, :, db:, :],
                        xv[:, :, : N - db, :],
                    ).then_inc(v_sem, 1)  # vb+2+k
                    if oi < len(own):
                        do_own(own[oi])
                        oi += 1
                while oi < len(own):
                    do_own(own[oi])
                    oi += 1
                nc.vector.memset(ot_a2[:], 0.0).then_inc(v_sem, 1)  # vb+V

        @block.scalar
        def _(scalar):
            for it in range(repeat):
                vb = V * it
                scalar.wait_ge(v_sem, vb + 1)
                nc.scalar.activation(dump_dif[:], dif[:], AFT.Abs, accum_out=a1[:])
                for k in range(NP):
                    scalar.wait_ge(v_sem, vb + 2 + k)
                    nc.scalar.activation(
                        dump[:],
                        ring[k % RING][:, :, : N - 1, :],
                        AFT.Copy,
                        accum_out=at[:, k + 1, :],
                    ).then_inc(s_sem, 1)
                scalar.wait_ge(v_sem, vb + V)  # all DVE adds done
                nc.scalar.activation(
                    dump[:], av, AFT.Copy, accum_out=at[:, 0, :]
                ).then_inc(s_sem, 1)
            nc.scalar.activation(
                at_sink[:], at[:].squeeze(2), AFT.Copy, accum_out=ot_a2[:]
            )
            nc.scalar.copy(ot[:, 0:1], a1[:])
            nc.scalar.copy(ot[:, 1:2], ot_a2[:]).then_inc(s_sem, 1)

    _NC_CACHE[key] = nc
    return nc


def _prep_inputs(predictions, targets):
    """Full f32 [B,N,H,W]/[B,H,W] -> per-core bf16 maps, layout [h,b,n,w]."""
    p = np.asarray(predictions, dtype=np.float32)
    t = np.asarray(targets, dtype=np.float32)
    pt = np.ascontiguousarray(p.transpose(2, 0, 1, 3)).astype(ml_dtypes.bfloat16)
    tt = np.ascontiguousarray(t.transpose(1, 0, 2)).astype(ml_dtypes.bfloat16)
    in_maps = []
    for c in range(N_CORES):
        xc = np.ascontiguousarray(pt[:, B_LOC * c : B_LOC * (c + 1)]).reshape(
            H, B_LOC * N * W
        )
        yc = np.ascontiguousarray(tt[:, B_LOC * c : B_LOC * (c + 1)]).reshape(
            H, B_LOC * W
        )
        in_maps.append({"x": xc, "y": yc})
    return in_maps


def _lat_weights_f64():
    lats = np.arange(90.0, -91.5, -1.5)  # [121]
    w = np.cos(np.deg2rad(lats))
    return H * (w / np.sum(w))


def _combine(outs, predictions):
    """outs: list of [H,2] f32 -> scalar f32 (host math in f64)."""
    w = _lat_weights_f64()
    p = np.asarray(predictions, dtype=np.float32)
    pb = p.astype(ml_dtypes.bfloat16).astype(np.float64)  # match device rounding
    a3_h = pb.sum(axis=(0, 1, 3))  # [H]
    a1_h = np.zeros(H, np.float64)
    a2_h = np.zeros(H, np.float64)
    for o in outs:
        o = np.asarray(o, dtype=np.float64)
        a1_h += o[:, 0]
        a2_h += o[:, 1]
    s2 = 2.0 * a2_h - (N - 1) * a3_h
    crps_h = a1_h / N - s2 / (N * N)
    total = float(np.dot(w, crps_h))
    return np.float32(total / (B * H * W))


def kernel(predictions, targets):
    nc = build_nc()
    in_maps = _prep_inputs(predictions, targets)
    res = run_bass_kernel_spmd(nc, in_maps, list(range(N_CORES)))
    outs = [res.results[i]["o"] for i in range(N_CORES)]
    return _combine(outs, predictions)
